# revision 1
# baseline (speedup 1.0000x reference)
"""Trainium2 Bass kernel for nn_MABSINK (multi-head attention w/ 1-step Sinkhorn,
residuals, LayerNorms, fused output MLP).

Sharding: tensor-parallel over heads (8 heads -> 8 cores) for projections +
attention; row-parallel (4096 rows -> 512/core) for LN0 + fc_o + LN1.
The head->row reshuffle happens on host between two device dispatches.

Math reformulation (eps=1, values bounded so exp is safe without max-sub):
  E = exp(S), arec[n] = 1/sum_m E[n,m]  (exp(u) = mu*arec)
  c'[m] = sum_n E[n,m]*arec[n]          (exp(vpot) = (nu/mu)/c')
  o = q + (mu*arec[n]) * (E @ (v * (nu/mu)/c'))    [nu == mu here]
Everything is computed in the transposed orientation ET[m,n] so the PE never
needs an on-chip transpose of the big matrices; Q^T/K^T are prepared host-side.
"""

import functools
import math

import ml_dtypes
import numpy as np

B, N, D, H, DH = 4, 1024, 1024, 8, 128
MU = 1.0 / N + 1e-8  # == nu
LN_EPS = 1e-5
SCALE = 1.0 / math.sqrt(D)  # 1/32, folded into Wk/bk host-side
NCORES = 8

LAST_EXEC_NS = {"d1": None, "d2": None}


def _mk_nc():
    import concourse.bacc as bacc

    return bacc.Bacc(
        "TRN2",
        target_bir_lowering=False,
        debug=False,
        enable_asserts=False,
        num_devices=NCORES,
    )


@functools.cache
def _build_d1(stage=5):
    """Dispatch 1: projections + sinkhorn attention for one head (= one core).

    out[b] = (q + attn)^T  as [DH, N] per batch.  stage<5 truncates for debug."""
    import concourse.bass as bass  # noqa: F401
    import concourse.mybir as mybir
    import concourse.tile as tile

    f32 = mybir.dt.float32
    f32r = mybir.dt.float32r
    bf16 = mybir.dt.bfloat16
    AF = mybir.ActivationFunctionType
    ALU = mybir.AluOpType

    nc = _mk_nc()
    QT = nc.dram_tensor("QT", [B, D, N], f32r, kind="ExternalInput").ap()
    KT = nc.dram_tensor("KT", [B, D, N], f32r, kind="ExternalInput").ap()
    WQ = nc.dram_tensor("WQ", [128, 8, 128], f32r, kind="ExternalInput").ap()
    WK = nc.dram_tensor("WK", [128, 8, 128], f32r, kind="ExternalInput").ap()
    WV = nc.dram_tensor("WV", [128, 8, 128], f32r, kind="ExternalInput").ap()
    BQ = nc.dram_tensor("BQ", [128, 1], f32, kind="ExternalInput").ap()
    BK = nc.dram_tensor("BK", [128, 1], f32, kind="ExternalInput").ap()
    BV = nc.dram_tensor("BV", [128, 1], f32, kind="ExternalInput").ap()
    ONESB = nc.dram_tensor("ONESB", [128, 1], bf16, kind="ExternalInput").ap()
    IDENT = nc.dram_tensor("IDENT", [128, 128], f32, kind="ExternalInput").ap()
    OT = nc.dram_tensor("OT", [B, DH, N], f32, kind="ExternalOutput").ap()

    with tile.TileContext(nc) as tc:
        with (
            tc.tile_pool(name="const", bufs=1) as constp,
            tc.tile_pool(name="io", bufs=3) as iop,
            tc.tile_pool(name="big", bufs=2) as bigp,
            tc.tile_pool(name="ktres", bufs=1) as ktp,
            tc.tile_pool(name="dramp", bufs=2, space="DRAM") as dramp,
            tc.tile_pool(name="ps_proj", bufs=2, space="PSUM") as ps_proj,
            tc.tile_pool(name="ps_st", bufs=2, space="PSUM") as ps_st,
            tc.tile_pool(name="ps_r", bufs=1, space="PSUM") as ps_r,
            tc.tile_pool(name="ps_ot", bufs=1, space="PSUM") as ps_ot,
            tc.tile_pool(name="ps_tr", bufs=2, space="PSUM") as ps_tr,
        ):
            wq = constp.tile([128, 8, 128], f32r)
            wk = constp.tile([128, 8, 128], f32r)
            wv = constp.tile([128, 8, 128], f32r)
            bq = constp.tile([128, 1], f32)
            bk = constp.tile([128, 1], f32)
            bv = constp.tile([128, 1], f32)
            ones = constp.tile([128, 1], bf16)
            ident = constp.tile([128, 128], f32)
            nc.sync.dma_start(wq, WQ)
            nc.sync.dma_start(wk, WK)
            nc.sync.dma_start(wv, WV)
            nc.sync.dma_start(bq, BQ)
            nc.sync.dma_start(bk, BK)
            nc.sync.dma_start(bv, BV)
            nc.sync.dma_start(ones, ONESB)
            nc.sync.dma_start(ident, IDENT)

            for b in range(B):
                # ---- resident KT[b] (used twice: k-proj and v-proj) ----
                ktc = ktp.tile([128, 8, N], f32r, tag="ktc")
                for kc in range(8):
                    nc.sync.dma_start(
                        ktc[:, kc, :], KT[b, kc * 128 : (kc + 1) * 128, :]
                    )

                # ---- q projection: qT[d, n] = Wq^T @ Q^T, streamed QT ----
                qps = [ps_proj.tile([128, 512], f32, tag="proj", name=f"qps{b}_{i}") for i in range(2)]
                for kc in range(8):
                    qtc = iop.tile([128, N], f32r, tag="qtc")
                    nc.sync.dma_start(qtc, QT[b, kc * 128 : (kc + 1) * 128, :])
                    for hf in range(2):
                        nc.tensor.matmul(
                            qps[hf],
                            wq[:, kc, :],
                            qtc[:, hf * 512 : (hf + 1) * 512],
                            start=(kc == 0),
                            stop=(kc == 7),
                        )
                qT = bigp.tile([128, N], f32r, tag="qT")
                for hf in range(2):
                    nc.scalar.activation(
                        qT[:, hf * 512 : (hf + 1) * 512], qps[hf], AF.Identity, bias=bq
                    )

                # ---- k projection (pre-scaled by 1/sqrt(D) host-side) ----
                kps = [ps_proj.tile([128, 512], f32, tag="proj", name=f"kps{b}_{i}") for i in range(2)]
                for kc in range(8):
                    for hf in range(2):
                        nc.tensor.matmul(
                            kps[hf],
                            wk[:, kc, :],
                            ktc[:, kc, hf * 512 : (hf + 1) * 512],
                            start=(kc == 0),
                            stop=(kc == 7),
                        )
                kT = bigp.tile([128, N], f32r, tag="kT")
                for hf in range(2):
                    nc.scalar.activation(
                        kT[:, hf * 512 : (hf + 1) * 512], kps[hf], AF.Identity, bias=bk
                    )

                # ---- v projection -> vT, then PE-transpose to v[m, d] ----
                vps = [ps_proj.tile([128, 512], f32, tag="proj", name=f"vps{b}_{i}") for i in range(2)]
                for kc in range(8):
                    for hf in range(2):
                        nc.tensor.matmul(
                            vps[hf],
                            wv[:, kc, :],
                            ktc[:, kc, hf * 512 : (hf + 1) * 512],
                            start=(kc == 0),
                            stop=(kc == 7),
                        )
                vT = bigp.tile([128, N], f32, tag="vT")
                for hf in range(2):
                    nc.scalar.activation(
                        vT[:, hf * 512 : (hf + 1) * 512], vps[hf], AF.Identity, bias=bv
                    )
                v = bigp.tile([128, 8, 128], f32, tag="v")
                for mt in range(8):
                    tp = ps_tr.tile([128, 128], f32, tag="tr")
                    nc.tensor.transpose(tp, vT[:, mt * 128 : (mt + 1) * 128], ident)
                    nc.scalar.activation(v[:, mt, :], tp, AF.Copy)

                if stage < 2:
                    nc.sync.dma_start(OT[b], qT.bitcast(f32))
                    continue
                # ---- ST = kT' @ qT blocks -> ET = exp(ST) (bf16) ----
                ET = bigp.tile([128, 8, N], bf16, tag="ET")
                for mt in range(8):
                    for hf in range(2):
                        sps = ps_st.tile([128, 512], f32, tag="st")
                        nc.tensor.matmul(
                            sps,
                            kT[:, mt * 128 : (mt + 1) * 128],
                            qT[:, hf * 512 : (hf + 1) * 512],
                            start=True,
                            stop=True,
                        )
                        nc.scalar.activation(
                            ET[:, mt, hf * 512 : (hf + 1) * 512], sps, AF.Exp
                        )

                if stage < 3:
                    nc.sync.dma_start(OT[b], qT.bitcast(f32))
                    continue
                # ---- arec[n] = 1 / sum_m ET[m, n]  (ones-matmul partition sum)
                arec = bigp.tile([1, N], f32, tag="arec")
                for hf in range(2):
                    rps = ps_r.tile([1, 512], f32, tag="r")
                    for mt in range(8):
                        nc.tensor.matmul(
                            rps,
                            ones,
                            ET[:, mt, hf * 512 : (hf + 1) * 512],
                            start=(mt == 0),
                            stop=(mt == 7),
                        )
                    nc.vector.reciprocal(arec[:, hf * 512 : (hf + 1) * 512], rps)
                abc = bigp.tile([128, N], f32, tag="abc")
                arecd = dramp.tile([1, N], f32, tag="arecd")
                nc.sync.dma_start(arecd, arec)
                nc.sync.dma_start(abc, arecd.broadcast_to([128, N]))

                if stage < 4:
                    nc.sync.dma_start(OT[b], abc)
                    continue
                # ---- c'[m] = sum_n ET[m, n] * arec[n] (fused DVE reduce) ----
                cw = bigp.tile([128, 8], f32, tag="cw")
                for mt in range(8):
                    ctmp = bigp.tile([128, N], bf16, tag="ctmp")
                    nc.vector.scalar_tensor_tensor(
                        out=ctmp,
                        in0=ET[:, mt, :],
                        scalar=1.0,
                        in1=abc,
                        op0=ALU.mult,
                        op1=ALU.mult,
                    )
                    nc.vector.tensor_reduce(
                        cw[:, mt : mt + 1], ctmp, mybir.AxisListType.X, ALU.add
                    )
                wrec = bigp.tile([128, 8], f32, tag="wrec")
                nc.vector.reciprocal(wrec, cw)

                # ---- vtilde = v * (1/c') (bf16) ----
                vv = bigp.tile([128, 8, 128], bf16, tag="vv")
                for mt in range(8):
                    nc.vector.tensor_scalar(
                        out=vv[:, mt, :],
                        in0=v[:, mt, :],
                        scalar1=wrec[:, mt : mt + 1],
                        scalar2=None,
                        op0=ALU.mult,
                    )

                if stage < 5:
                    nc.sync.dma_start(OT[b], abc)
                    continue
                # ---- oT = vtilde^T @ ET; o = mu*arec*oT + qT ----
                ofinT = bigp.tile([128, N], f32, tag="ofinT")
                for hf in range(2):
                    ops_ = ps_ot.tile([128, 512], f32, tag="ot")
                    for mt in range(8):
                        nc.tensor.matmul(
                            ops_,
                            vv[:, mt, :],
                            ET[:, mt, hf * 512 : (hf + 1) * 512],
                            start=(mt == 0),
                            stop=(mt == 7),
                        )
                    hs = slice(hf * 512, (hf + 1) * 512)
                    nc.vector.scalar_tensor_tensor(
                        out=ofinT[:, hs],
                        in0=ops_,
                        scalar=float(MU),
                        in1=abc[:, hs],
                        op0=ALU.mult,
                        op1=ALU.mult,
                    )
                    nc.vector.tensor_tensor(
                        ofinT[:, hs], ofinT[:, hs], qT[:, hs].bitcast(f32), ALU.add
                    )
                nc.sync.dma_start(OT[b], ofinT)

    nc.compile()
    return nc


@functools.cache
def _build_d2():
    """Dispatch 2: LN0 -> fc_o(+relu, residual) -> LN1 on 512 rows per core."""
    import concourse.mybir as mybir
    import concourse.tile as tile

    f32 = mybir.dt.float32
    f32r = mybir.dt.float32r
    AF = mybir.ActivationFunctionType
    ALU = mybir.AluOpType

    nc = _mk_nc()
    OIN = nc.dram_tensor("OIN", [4, 128, D], f32, kind="ExternalInput").ap()
    WO = nc.dram_tensor("WO", [8, 128, D], f32r, kind="ExternalInput").ap()
    BOBC = nc.dram_tensor("BOBC", [128, D], f32, kind="ExternalInput").ap()
    G0BC = nc.dram_tensor("G0BC", [128, D], f32, kind="ExternalInput").ap()
    BE0BC = nc.dram_tensor("BE0BC", [128, D], f32, kind="ExternalInput").ap()
    G1BC = nc.dram_tensor("G1BC", [128, D], f32, kind="ExternalInput").ap()
    BE1BC = nc.dram_tensor("BE1BC", [128, D], f32, kind="ExternalInput").ap()
    IDENT = nc.dram_tensor("IDENT", [128, 128], f32, kind="ExternalInput").ap()
    OUT2 = nc.dram_tensor("OUT2", [4, 128, D], f32, kind="ExternalOutput").ap()

    with tile.TileContext(nc) as tc:
        with (
            tc.tile_pool(name="const", bufs=1) as constp,
            tc.tile_pool(name="work", bufs=2) as wp,
            tc.tile_pool(name="small", bufs=4) as sp,
            tc.tile_pool(name="ps_mm", bufs=2, space="PSUM") as ps_mm,
            tc.tile_pool(name="ps_tr", bufs=2, space="PSUM") as ps_tr,
        ):
            wo = constp.tile([128, 8, D], f32r)
            nc.sync.dma_start(wo, WO.rearrange("kc p n -> p kc n"))
            bobc = constp.tile([128, D], f32)
            g0bc = constp.tile([128, D], f32)
            be0bc = constp.tile([128, D], f32)
            g1bc = constp.tile([128, D], f32)
            be1bc = constp.tile([128, D], f32)
            ident = constp.tile([128, 128], f32)
            nc.sync.dma_start(bobc, BOBC)
            nc.sync.dma_start(g0bc, G0BC)
            nc.sync.dma_start(be0bc, BE0BC)
            nc.sync.dma_start(g1bc, G1BC)
            nc.sync.dma_start(be1bc, BE1BC)
            nc.sync.dma_start(ident, IDENT)

            def layernorm(x_in, g_bc, be_bc, out_tag):
                """x: [128, D] f32 -> normalized * g + be."""
                msum = sp.tile([128, 1], f32, tag="msum")
                junk = wp.tile([128, D], f32, tag="junk")
                nc.scalar.activation(junk, x_in, AF.Copy, accum_out=msum)
                mneg = sp.tile([128, 1], f32, tag="mneg")
                nc.vector.tensor_scalar(
                    out=mneg, in0=msum, scalar1=-1.0 / D, scalar2=None, op0=ALU.mult
                )
                xm = wp.tile([128, D], f32, tag="xm")
                nc.vector.tensor_scalar(
                    out=xm, in0=x_in, scalar1=mneg, scalar2=None, op0=ALU.add
                )
                vsum = sp.tile([128, 1], f32, tag="vsum")
                junk2 = wp.tile([128, D], f32, tag="junk")
                nc.scalar.activation(junk2, xm, AF.Square, accum_out=vsum)
                sv = sp.tile([128, 1], f32, tag="sv")
                nc.vector.tensor_scalar(
                    out=sv,
                    in0=vsum,
                    scalar1=1.0 / D,
                    scalar2=LN_EPS,
                    op0=ALU.mult,
                    op1=ALU.add,
                )
                sq = sp.tile([128, 1], f32, tag="sq")
                nc.scalar.activation(sq, sv, AF.Sqrt)
                rstd = sp.tile([128, 1], f32, tag="rstd")
                nc.vector.reciprocal(rstd, sq)
                o = wp.tile([128, D], f32, tag=out_tag)
                nc.vector.scalar_tensor_tensor(
                    out=o, in0=xm, scalar=rstd, in1=g_bc, op0=ALU.mult, op1=ALU.mult
                )
                nc.vector.tensor_tensor(o, o, be_bc, ALU.add)
                return o

            for t in range(4):
                ob = wp.tile([128, D], f32, tag="ob")
                nc.sync.dma_start(ob, OIN[t])
                oln = layernorm(ob, g0bc, be0bc, "oln")

                olnT = wp.tile([128, 8, 128], f32r, tag="olnT")
                for dc in range(8):
                    tp = ps_tr.tile([128, 128], f32, tag="tr")
                    nc.tensor.transpose(tp, oln[:, dc * 128 : (dc + 1) * 128], ident)
                    nc.scalar.activation(olnT[:, dc, :], tp, AF.Copy)

                t1 = wp.tile([128, D], f32, tag="t1")
                for hf in range(2):
                    o1ps = ps_mm.tile([128, 512], f32, tag="mm")
                    for dc in range(8):
                        nc.tensor.matmul(
                            o1ps,
                            olnT[:, dc, :],
                            wo[:, dc, hf * 512 : (hf + 1) * 512],
                            start=(dc == 0),
                            stop=(dc == 7),
                        )
                    hs = slice(hf * 512, (hf + 1) * 512)
                    nc.vector.tensor_tensor(t1[:, hs], o1ps, bobc[:, hs], ALU.add)
                r1 = wp.tile([128, D], f32, tag="r1")
                nc.scalar.activation(r1, t1, AF.Relu)
                o2 = wp.tile([128, D], f32, tag="o2")
                nc.vector.tensor_tensor(o2, r1, oln, ALU.add)

                o3 = layernorm(o2, g1bc, be1bc, "o3")
                nc.sync.dma_start(OUT2[t], o3)

    nc.compile()
    return nc


def _run(nc, in_maps, trace=False):
    from concourse.bass_utils import run_bass_kernel_spmd

    return run_bass_kernel_spmd(
        nc, in_maps, list(range(NCORES)), trace=trace
    )


def kernel(**inputs):
    trace = bool(int(__import__("os").environ.get("KERNEL_TRACE", "0")))
    f32 = np.float32
    Q = np.ascontiguousarray(inputs["Q"], dtype=f32)
    K = np.ascontiguousarray(inputs["K"], dtype=f32)
    Wq, Wk, Wv, Wo = (np.asarray(inputs[k], f32) for k in ("Wq", "Wk", "Wv", "Wo"))
    bq, bk, bv, bo = (np.asarray(inputs[k], f32) for k in ("bq", "bk", "bv", "bo"))
    g0, be0, g1, be1 = (np.asarray(inputs[k], f32) for k in ("g0", "be0", "g1", "be1"))

    QT = np.ascontiguousarray(Q.transpose(0, 2, 1))
    KT = np.ascontiguousarray(K.transpose(0, 2, 1))
    ident = np.eye(128, dtype=f32)
    onesb = np.ones((128, 1), dtype=ml_dtypes.bfloat16)

    in_maps = []
    for h in range(H):
        hs = slice(h * DH, (h + 1) * DH)
        wqh = np.ascontiguousarray(
            Wq[:, hs].reshape(8, 128, 128).transpose(1, 0, 2)
        )
        wkh = np.ascontiguousarray(
            (Wk[:, hs] * SCALE).reshape(8, 128, 128).transpose(1, 0, 2)
        )
        wvh = np.ascontiguousarray(
            Wv[:, hs].reshape(8, 128, 128).transpose(1, 0, 2)
        )
        in_maps.append(
            {
                "QT": QT,
                "KT": KT,
                "WQ": wqh,
                "WK": wkh,
                "WV": wvh,
                "BQ": bq[hs].reshape(128, 1).astype(f32),
                "BK": (bk[hs] * SCALE).reshape(128, 1).astype(f32),
                "BV": bv[hs].reshape(128, 1).astype(f32),
                "ONESB": onesb,
                "IDENT": ident,
            }
        )

    r1 = _run(_build_d1(), in_maps, trace=trace)
    LAST_EXEC_NS["d1"] = r1.exec_time_ns

    # assemble O_attn [B, N, DV] from per-head oT outputs [B, DH, N]
    O_attn = np.empty((B, N, D), dtype=f32)
    for h in range(H):
        O_attn[:, :, h * DH : (h + 1) * DH] = r1.results[h]["OT"].transpose(0, 2, 1)

    O_flat = O_attn.reshape(B * N, D)
    wo_in = np.ascontiguousarray(Wo.reshape(8, 128, D))
    bc = lambda x: np.ascontiguousarray(np.broadcast_to(x, (128, D)), dtype=f32)
    in_maps2 = []
    for c in range(NCORES):
        in_maps2.append(
            {
                "OIN": np.ascontiguousarray(
                    O_flat[c * 512 : (c + 1) * 512].reshape(4, 128, D)
                ),
                "WO": wo_in,
                "BOBC": bc(bo),
                "G0BC": bc(g0),
                "BE0BC": bc(be0),
                "G1BC": bc(g1),
                "BE1BC": bc(be1),
                "IDENT": ident,
            }
        )
    r2 = _run(_build_d2(), in_maps2, trace=trace)
    LAST_EXEC_NS["d2"] = r2.exec_time_ns

    out = np.empty((B * N, D), dtype=f32)
    for c in range(NCORES):
        out[c * 512 : (c + 1) * 512] = r2.results[c]["OUT2"].reshape(512, D)
    return out.reshape(B, N, D)



# revision 14
# speedup vs baseline: 1.2635x; 1.2635x over previous
"""Trainium2 Bass kernel for nn_MABSINK (multi-head attention w/ 1-step Sinkhorn,
residuals, LayerNorms, fused output MLP).

Sharding: tensor-parallel over heads (8 heads -> 8 cores) for projections +
attention (dispatch 1); column-parallel over (batch, n-half) for LN0 + fc_o +
LN1 in the TRANSPOSED orientation (dispatch 2) so no on-chip transposes are
needed anywhere in dispatch 2 (d1's outputs are already [d, n]).

Math (eps=1, mu=nu):
  E = exp(S^T) stored [m, n];  R[n] = sum_m E  (fp8 DoubleRow ones-matmul)
  abc = 64/R broadcast;  ETa = E*abc (in-place, accum_out -> cw = 64*c)
  vv = v * (64/cw);  attn^T = ETa^T-contraction (fp8 DoubleRow matmul)
  o^T = (MU/64)*attn + q^T
Dispatch 2 (transposed): LN stats via gpsimd partition_all_reduce, fc_o with
Wo-chunk weights (contraction over d = partitions), fused bias+relu in PSUM
evacuation.
"""

import functools
import math

import ml_dtypes
import numpy as np

B, N, D, H, DH = 4, 1024, 1024, 8, 128
MU = 1.0 / N + 1e-8  # == nu
LN_EPS = 1e-5
SCALE = 1.0 / math.sqrt(D)  # 1/32
NCORES = 8
OMEGA = 1.0 / 128.0  # fp8-range shift: ETa = E/(omega*R) lands in [0.03, 0.7]

LAST_EXEC_NS = {"d1": None, "d2": None}


def _mk_nc():
    import concourse.bacc as bacc

    return bacc.Bacc(
        "TRN2",
        target_bir_lowering=False,
        debug=False,
        enable_asserts=False,
        num_devices=NCORES,
    )


@functools.cache
def _build_d1():
    """Dispatch 1: projections + sinkhorn attention for one head (= one core).

    out[b] = (q + attn)^T as [DH, N] bf16 per batch."""
    import concourse.bass as bass  # noqa: F401
    import concourse.mybir as mybir
    import concourse.tile as tile

    f32 = mybir.dt.float32
    bf16 = mybir.dt.bfloat16
    f8 = mybir.dt.float8e4
    AF = mybir.ActivationFunctionType
    ALU = mybir.AluOpType
    DR = mybir.MatmulPerfMode.DoubleRow

    nc = _mk_nc()
    QT = nc.dram_tensor("QT", [B, D, N], bf16, kind="ExternalInput").ap()
    KT8 = nc.dram_tensor("KT8", [B, 8, 128, N], f8, kind="ExternalInput").ap()
    WQ = nc.dram_tensor("WQ", [128, 8, 128], bf16, kind="ExternalInput").ap()
    WK = nc.dram_tensor("WK", [128, 8, 128], f8, kind="ExternalInput").ap()
    WV = nc.dram_tensor("WV", [128, 8, 128], f8, kind="ExternalInput").ap()
    BQ = nc.dram_tensor("BQ", [128, 1], f32, kind="ExternalInput").ap()
    BK = nc.dram_tensor("BK", [128, 1], f32, kind="ExternalInput").ap()
    BV = nc.dram_tensor("BV", [128, 1], f32, kind="ExternalInput").ap()
    ONES8 = nc.dram_tensor("ONES8", [128, 2, 128], f8, kind="ExternalInput").ap()
    OT = nc.dram_tensor("OT", [B, DH, N], f32, kind="ExternalOutput").ap()

    with tile.TileContext(nc) as tc:
        with (
            tc.tile_pool(name="const", bufs=1) as constp,
            tc.tile_pool(name="io", bufs=3) as iop,
            tc.tile_pool(name="kt", bufs=2) as ktp,
            tc.tile_pool(name="mid", bufs=2) as midp,
            tc.tile_pool(name="et", bufs=2) as etp,
            tc.tile_pool(name="ps_proj", bufs=2, space="PSUM") as ps_proj,
            tc.tile_pool(name="ps_st", bufs=2, space="PSUM") as ps_st,
            tc.tile_pool(name="ps_r", bufs=2, space="PSUM") as ps_r,
            tc.tile_pool(name="ps_ot", bufs=2, space="PSUM") as ps_ot,
        ):
            wq = constp.tile([128, 8, 128], bf16)
            wk = constp.tile([128, 8, 128], f8)
            wv = constp.tile([128, 8, 128], f8)
            bq = constp.tile([128, 1], f32)
            bk = constp.tile([128, 1], f32)
            bv = constp.tile([128, 1], f32)
            ones8 = constp.tile([128, 2, 128], f8)
            nc.sync.dma_start(wq, WQ)
            nc.sync.dma_start(wk, WK)
            nc.sync.dma_start(wv, WV)
            nc.sync.dma_start(bq, BQ)
            nc.sync.dma_start(bk, BK)
            nc.sync.dma_start(bv, BV)
            nc.sync.dma_start(ones8, ONES8)

            for b in range(B):
                # ---- resident KT[b] fp8 (used by k-proj and v-proj) ----
                ktc = ktp.tile([128, 8, N], f8, tag="ktc")
                nc.sync.dma_start(ktc, KT8[b].rearrange("kc p n -> p kc n"))

                # ---- q projection (bf16): qT = Wq^T @ Q^T + bq ----
                qps = [
                    ps_proj.tile([128, 512], f32, tag="proj", name=f"qps{b}_{i}")
                    for i in range(2)
                ]
                for kc in range(8):
                    qtc = iop.tile([128, N], bf16, tag="qtc")
                    nc.sync.dma_start(qtc, QT[b, kc * 128 : (kc + 1) * 128, :])
                    for hf in range(2):
                        nc.tensor.matmul(
                            qps[hf],
                            wq[:, kc, :],
                            qtc[:, hf * 512 : (hf + 1) * 512],
                            start=(kc == 0),
                            stop=(kc == 7),
                        )
                qT = midp.tile([128, N], bf16, tag="qT")
                qTf = midp.tile([128, N], f32, tag="qTf")
                for hf in range(2):
                    hs = slice(hf * 512, (hf + 1) * 512)
                    nc.scalar.activation(qTf[:, hs], qps[hf], AF.Identity, bias=bq)
                    with nc.allow_low_precision(reason="bf16 q for S matmul"):
                        nc.vector.tensor_copy(qT[:, hs], qTf[:, hs])

                # ---- k projection fp8 DoubleRow (weights pre-scaled x32,
                #      S-scale 1/32 folded => evac scale 1/1024) ----
                kps = [
                    ps_proj.tile([128, 512], f32, tag="proj", name=f"kps{b}_{i}")
                    for i in range(2)
                ]
                for t in range(4):
                    for hf in range(2):
                        nc.tensor.matmul(
                            kps[hf],
                            wk[:, 2 * t : 2 * t + 2, :],
                            ktc[:, 2 * t : 2 * t + 2, hf * 512 : (hf + 1) * 512],
                            start=(t == 0),
                            stop=(t == 3),
                            perf_mode=DR,
                        )
                kT = midp.tile([128, N], bf16, tag="kT")
                for hf in range(2):
                    nc.scalar.activation(
                        kT[:, hf * 512 : (hf + 1) * 512],
                        kps[hf],
                        AF.Identity,
                        bias=bk,
                        scale=1.0 / 1024.0,
                    )

                # ---- v projection fp8 DoubleRow (weights x32 => evac 1/32) ----
                vps = [
                    ps_proj.tile([128, 512], f32, tag="proj", name=f"vps{b}_{i}")
                    for i in range(2)
                ]
                for t in range(4):
                    for hf in range(2):
                        nc.tensor.matmul(
                            vps[hf],
                            wv[:, 2 * t : 2 * t + 2, :],
                            ktc[:, 2 * t : 2 * t + 2, hf * 512 : (hf + 1) * 512],
                            start=(t == 0),
                            stop=(t == 3),
                            perf_mode=DR,
                        )
                vT = midp.tile([128, N], bf16, tag="vT")
                for hf in range(2):
                    nc.scalar.activation(
                        vT[:, hf * 512 : (hf + 1) * 512],
                        vps[hf],
                        AF.Identity,
                        bias=bv,
                        scale=1.0 / 32.0,
                    )
                # ---- v = vT^T via DMA xbar transpose (offloads PE) ----
                v = midp.tile([128, 8, 128], bf16, tag="v")
                for mt in range(8):
                    nc.sync.dma_start_transpose(
                        v[:, mt, :], vT[:, mt * 128 : (mt + 1) * 128]
                    )

                # ---- ST = kT' @ qT blocks -> ET = exp(ST) (fp8) ----
                ET = etp.tile([128, 8, N], f8, tag="ET")
                for mt in range(8):
                    for hf in range(2):
                        sps = ps_st.tile([128, 512], f32, tag="st")
                        nc.tensor.matmul(
                            sps,
                            kT[:, mt * 128 : (mt + 1) * 128],
                            qT[:, hf * 512 : (hf + 1) * 512],
                            start=True,
                            stop=True,
                        )
                        nc.scalar.activation(
                            ET[:, mt, hf * 512 : (hf + 1) * 512], sps, AF.Exp
                        )

                # ---- omega*R[n] via fp8 DR all-ones matmul: every PSUM
                #      partition gets the same colsum => broadcast for free ----
                abc = midp.tile([128, N], bf16, tag="abc")
                for hf in range(2):
                    rps = ps_r.tile([128, 512], f32, tag="r", name=f"rps{b}_{hf}")
                    for t in range(4):
                        nc.tensor.matmul(
                            rps,
                            ones8,
                            ET[:, 2 * t : 2 * t + 2, hf * 512 : (hf + 1) * 512],
                            start=(t == 0),
                            stop=(t == 3),
                            perf_mode=DR,
                        )
                    with nc.allow_low_precision(reason="bf16 arec is plenty"):
                        nc.vector.reciprocal(abc[:, hf * 512 : (hf + 1) * 512], rps)

                # ---- ETa = ET * abc in place; cw[m] = sum_n ETa (accum) ----
                cw = midp.tile([128, 8], f32, tag="cw")
                for mt in range(8):
                    nc.vector.scalar_tensor_tensor(
                        out=ET[:, mt, :],
                        in0=ET[:, mt, :],
                        scalar=1.0,
                        in1=abc,
                        op0=ALU.mult,
                        op1=ALU.mult,
                        accum_out=cw[:, mt : mt + 1],
                    )
                wrec = midp.tile([128, 8], f32, tag="wrec")
                nc.vector.reciprocal(wrec, cw)

                # ---- vv = v * (1/(omega*cw)) (fp8) ----
                vv = midp.tile([128, 8, 128], f8, tag="vv")
                for mt in range(8):
                    nc.vector.tensor_scalar(
                        out=vv[:, mt, :],
                        in0=v[:, mt, :],
                        scalar1=wrec[:, mt : mt + 1],
                        scalar2=1.0 / OMEGA,
                        op0=ALU.mult,
                        op1=ALU.mult,
                    )

                # ---- attn (fp8 DR): oT = vv^T @ ETa; out = MU/64*oT + qT ----
                ops_ = [
                    ps_ot.tile([128, 512], f32, tag="ot", name=f"ot{b}_{i}")
                    for i in range(2)
                ]
                for t in range(4):
                    for hf in range(2):
                        nc.tensor.matmul(
                            ops_[hf],
                            vv[:, 2 * t : 2 * t + 2, :],
                            ET[:, 2 * t : 2 * t + 2, hf * 512 : (hf + 1) * 512],
                            start=(t == 0),
                            stop=(t == 3),
                            perf_mode=DR,
                        )
                ofin = midp.tile([128, N], f32, tag="ofin")
                for hf in range(2):
                    hs = slice(hf * 512, (hf + 1) * 512)
                    nc.vector.scalar_tensor_tensor(
                        out=ofin[:, hs],
                        in0=ops_[hf],
                        scalar=float(MU) * OMEGA,
                        in1=qTf[:, hs],
                        op0=ALU.mult,
                        op1=ALU.add,
                    )
                nc.sync.dma_start(OT[b], ofin)

    nc.compile()
    return nc


@functools.cache
def _build_d2():
    """Dispatch 2 (transposed): LN0 -> fc_o(+relu, residual) -> LN1 on a
    [1024 d, 512 n] column slab per core. No on-chip transposes."""
    import concourse.bass_isa as bass_isa
    import concourse.mybir as mybir
    import concourse.tile as tile

    f32 = mybir.dt.float32
    bf16 = mybir.dt.bfloat16
    AF = mybir.ActivationFunctionType
    ALU = mybir.AluOpType
    ROP = bass_isa.ReduceOp

    NC_ = 256  # column tile (2 per core), pipelines LN (DVE) under fc_o (PE)

    nc = _mk_nc()
    XIN = nc.dram_tensor("XIN", [8, 128, 512], bf16, kind="ExternalInput").ap()
    WOT = nc.dram_tensor("WOT", [128, 8, D], bf16, kind="ExternalInput").ap()
    BO = nc.dram_tensor("BO", [128, 8], f32, kind="ExternalInput").ap()
    G0 = nc.dram_tensor("G0", [128, 8], f32, kind="ExternalInput").ap()
    BE0 = nc.dram_tensor("BE0", [128, 8], f32, kind="ExternalInput").ap()
    G1 = nc.dram_tensor("G1", [128, 8], f32, kind="ExternalInput").ap()
    BE1 = nc.dram_tensor("BE1", [128, 8], f32, kind="ExternalInput").ap()
    OUT2 = nc.dram_tensor("OUT2", [8, 128, 512], f32, kind="ExternalOutput").ap()

    with tile.TileContext(nc) as tc:
        with (
            tc.tile_pool(name="const", bufs=1) as constp,
            tc.tile_pool(name="work", bufs=2) as wp,
            tc.tile_pool(name="small", bufs=2) as sp,
            tc.tile_pool(name="ps_mm", bufs=4, space="PSUM") as ps_mm,
        ):
            wot = constp.tile([128, 8, D], bf16)
            boc = constp.tile([128, 8], f32)
            g0c = constp.tile([128, 8], f32)
            be0c = constp.tile([128, 8], f32)
            g1c = constp.tile([128, 8], f32)
            be1c = constp.tile([128, 8], f32)
            nc.sync.dma_start(wot, WOT)
            nc.sync.dma_start(boc, BO)
            nc.sync.dma_start(g0c, G0)
            nc.sync.dma_start(be0c, BE0)
            nc.sync.dma_start(g1c, G1)
            nc.sync.dma_start(be1c, BE1)

            def ln_tr(x_in, gcol, becol, out_dtype, out_tag):
                """x_in [128, 8, NC_]: LN over d = (partition, chunk)."""
                t4 = sp.tile([128, 4, NC_], bf16, tag="t4")
                nc.vector.tensor_tensor(t4, x_in[:, 0:4, :], x_in[:, 4:8, :], ALU.add)
                t2 = sp.tile([128, 2, NC_], bf16, tag="t2")
                nc.vector.tensor_tensor(t2, t4[:, 0:2, :], t4[:, 2:4, :], ALU.add)
                s1 = sp.tile([128, NC_], f32, tag="s1")
                nc.vector.tensor_tensor(s1, t2[:, 0, :], t2[:, 1, :], ALU.add)
                s1bc = sp.tile([128, NC_], f32, tag="s1bc")
                nc.gpsimd.partition_all_reduce(s1bc, s1, 128, ROP.add)

                xsq = sp.tile([128, 8, NC_], bf16, tag="xsq")
                nc.vector.tensor_tensor(xsq, x_in, x_in, ALU.mult)
                q4 = sp.tile([128, 4, NC_], bf16, tag="q4")
                nc.vector.tensor_tensor(q4, xsq[:, 0:4, :], xsq[:, 4:8, :], ALU.add)
                q2 = sp.tile([128, 2, NC_], bf16, tag="q2")
                nc.vector.tensor_tensor(q2, q4[:, 0:2, :], q4[:, 2:4, :], ALU.add)
                s2 = sp.tile([128, NC_], f32, tag="s2")
                nc.vector.tensor_tensor(s2, q2[:, 0, :], q2[:, 1, :], ALU.add)
                s2bc = sp.tile([128, NC_], f32, tag="s2bc")
                nc.gpsimd.partition_all_reduce(s2bc, s2, 128, ROP.add)

                mbc = sp.tile([128, NC_], f32, tag="mbc")
                nc.vector.tensor_scalar(
                    out=mbc, in0=s1bc, scalar1=1.0 / D, scalar2=None, op0=ALU.mult
                )
                m2 = sp.tile([128, NC_], f32, tag="m2")
                nc.vector.tensor_tensor(m2, mbc, mbc, ALU.mult)
                v1 = sp.tile([128, NC_], f32, tag="v1")
                nc.vector.tensor_scalar(
                    out=v1,
                    in0=s2bc,
                    scalar1=1.0 / D,
                    scalar2=LN_EPS,
                    op0=ALU.mult,
                    op1=ALU.add,
                )
                var = sp.tile([128, NC_], f32, tag="var")
                nc.vector.tensor_tensor(var, v1, m2, ALU.subtract)
                sq = sp.tile([128, NC_], f32, tag="sq")
                nc.scalar.activation(sq, var, AF.Sqrt)
                rstd = sp.tile([128, NC_], f32, tag="rstd")
                nc.vector.reciprocal(rstd, sq)
                mb16 = sp.tile([128, NC_], bf16, tag="mb16")
                with nc.allow_low_precision(reason="LN mean in bf16 is fine"):
                    nc.vector.tensor_copy(mb16, mbc)
                rb16 = sp.tile([128, NC_], bf16, tag="rb16")
                with nc.allow_low_precision(reason="LN rstd in bf16 is fine"):
                    nc.vector.tensor_copy(rb16, rstd)

                out = wp.tile([128, 8, NC_], out_dtype, tag=out_tag)
                for dc in range(8):
                    xc = sp.tile([128, NC_], bf16, tag="xc")
                    nc.vector.tensor_tensor(xc, x_in[:, dc, :], mb16, ALU.subtract)
                    xr = sp.tile([128, NC_], bf16, tag="xr")
                    nc.vector.tensor_tensor(xr, xc, rb16, ALU.mult)
                    nc.scalar.activation(
                        out[:, dc, :],
                        xr,
                        AF.Identity,
                        scale=gcol[:, dc : dc + 1],
                        bias=becol[:, dc : dc + 1],
                    )
                return out

            for cs in range(2):
                ns = slice(cs * NC_, (cs + 1) * NC_)
                x = wp.tile([128, 8, NC_], bf16, tag="x")
                nc.sync.dma_start(x, XIN[:, :, ns].rearrange("dc p n -> p dc n"))

                oln = ln_tr(x, g0c, be0c, bf16, "oln")

                t1 = wp.tile([128, 8, NC_], bf16, tag="t1")
                for do in range(8):
                    ps = ps_mm.tile([128, NC_], f32, tag="mm")
                    for di in range(8):
                        nc.tensor.matmul(
                            ps,
                            wot[:, di, do * 128 : (do + 1) * 128],
                            oln[:, di, :],
                            start=(di == 0),
                            stop=(di == 7),
                        )
                    nc.scalar.activation(
                        t1[:, do, :], ps, AF.Relu, bias=boc[:, do : do + 1]
                    )
                o2 = wp.tile([128, 8, NC_], bf16, tag="o2")
                nc.vector.tensor_tensor(o2, t1, oln, ALU.add)

                o3 = ln_tr(o2, g1c, be1c, f32, "o3")
                nc.sync.dma_start(
                    OUT2[:, :, ns].rearrange("dc p n -> p dc n"), o3
                )

    nc.compile()
    return nc


def _run(nc, in_maps, trace=False):
    from concourse.bass_utils import run_bass_kernel_spmd

    return run_bass_kernel_spmd(nc, in_maps, list(range(NCORES)), trace=trace)


def kernel(**inputs):
    trace = bool(int(__import__("os").environ.get("KERNEL_TRACE", "0")))
    f32 = np.float32
    bf16 = ml_dtypes.bfloat16
    f8 = ml_dtypes.float8_e4m3fn
    Q = np.ascontiguousarray(inputs["Q"], dtype=f32)
    K = np.ascontiguousarray(inputs["K"], dtype=f32)
    Wq, Wk, Wv, Wo = (np.asarray(inputs[k], f32) for k in ("Wq", "Wk", "Wv", "Wo"))
    bq, bk, bv, bo = (np.asarray(inputs[k], f32) for k in ("bq", "bk", "bv", "bo"))
    g0, be0, g1, be1 = (np.asarray(inputs[k], f32) for k in ("g0", "be0", "g1", "be1"))

    QT = np.ascontiguousarray(Q.transpose(0, 2, 1)).astype(bf16)
    KT8 = (
        np.ascontiguousarray(K.transpose(0, 2, 1))
        .reshape(B, 8, 128, N)
        .astype(f8)
    )
    ones8 = np.full((128, 2, 128), OMEGA, dtype=f8)

    in_maps = []
    for h in range(H):
        hs = slice(h * DH, (h + 1) * DH)
        wqh = np.ascontiguousarray(
            Wq[:, hs].reshape(8, 128, 128).transpose(1, 0, 2)
        ).astype(bf16)
        wkh = np.ascontiguousarray(
            (Wk[:, hs] * 32.0).reshape(8, 128, 128).transpose(1, 0, 2)
        ).astype(f8)
        wvh = np.ascontiguousarray(
            (Wv[:, hs] * 32.0).reshape(8, 128, 128).transpose(1, 0, 2)
        ).astype(f8)
        in_maps.append(
            {
                "QT": QT,
                "KT8": KT8,
                "WQ": wqh,
                "WK": wkh,
                "WV": wvh,
                "BQ": bq[hs].reshape(128, 1).astype(f32),
                "BK": (bk[hs] * SCALE).reshape(128, 1).astype(f32),
                "BV": bv[hs].reshape(128, 1).astype(f32),
                "ONES8": ones8,
            }
        )

    r1 = _run(_build_d1(), in_maps, trace=trace)
    LAST_EXEC_NS["d1"] = r1.exec_time_ns

    # ---- host reshard: per-head [B, DH, N] bf16 -> per-core [8, 128, 512] ----
    # core c <-> (batch c//2, n-half c%2); chunk dim = head index
    OTall = np.stack([r1.results[h]["OT"] for h in range(H)])  # [8, B, 128, N]
    wot_in = np.ascontiguousarray(
        Wo.reshape(8, 128, D).transpose(1, 0, 2)
    ).astype(bf16)
    col = lambda z: np.ascontiguousarray(z.reshape(8, 128).T, dtype=f32)
    boc, g0c, be0c, g1c, be1c = col(bo), col(g0), col(be0), col(g1), col(be1)
    in_maps2 = []
    for c in range(NCORES):
        b, nh = c // 2, c % 2
        xin = np.ascontiguousarray(
            OTall[:, b, :, nh * 512 : (nh + 1) * 512]
        ).astype(bf16)
        in_maps2.append(
            {
                "XIN": xin,
                "WOT": wot_in,
                "BO": boc,
                "G0": g0c,
                "BE0": be0c,
                "G1": g1c,
                "BE1": be1c,
            }
        )
    r2 = _run(_build_d2(), in_maps2, trace=trace)
    LAST_EXEC_NS["d2"] = r2.exec_time_ns

    # ---- host unshard: [8 dc, 128 p, 512 n] -> [n, d] rows of O ----
    out = np.empty((B, N, D), dtype=f32)
    for c in range(NCORES):
        b, nh = c // 2, c % 2
        slab = r2.results[c]["OUT2"]  # [8, 128, 512] f32
        out[b, nh * 512 : (nh + 1) * 512, :] = (
            slab.transpose(2, 0, 1).reshape(512, D)
        )
    return out


# revision 15
# speedup vs baseline: 1.4348x; 1.1356x over previous
"""Trainium2 Bass kernel for nn_MABSINK (multi-head attention w/ 1-step Sinkhorn,
residuals, LayerNorms, fused output MLP).

Sharding: tensor-parallel over heads (8 heads -> 8 cores) for projections +
attention (dispatch 1); column-parallel over (batch, n-half) for LN0 + fc_o +
LN1 in the TRANSPOSED orientation (dispatch 2) so no on-chip transposes are
needed anywhere in dispatch 2 (d1's outputs are already [d, n]).

Both dispatches are software-pipelined: per-engine program order is interleaved
across batch/column tiles so the PE never sits behind a vector-chain of the
previous tile.

d1 math (eps=1, mu=nu):
  E = exp(S^T) stored [m, n] fp8;  omega*R[n] broadcast to all PSUM partitions
  via an all-ones fp8 DoubleRow matmul;  abc = 1/(omega*R) (fast reciprocal)
  ETa = E*abc in-place (accum_out -> cw);  vv = v/(omega*cw)
  attn^T via fp8 DoubleRow matmul;  o^T = MU*omega*attn + q^T (f32 residual)
d2: LN stats via broadcast all-ones bf16 matmuls (PE), apply on DVE/Scalar,
fc_o with transposed Wo chunks, fused bias+relu in PSUM evacuation.
"""

import functools
import math

import ml_dtypes
import numpy as np

B, N, D, H, DH = 4, 1024, 1024, 8, 128
MU = 1.0 / N + 1e-8  # == nu
LN_EPS = 1e-5
SCALE = 1.0 / math.sqrt(D)  # 1/32
NCORES = 8
OMEGA = 1.0 / 128.0  # fp8-range shift: ETa = E/(omega*R) lands in [0.03, 0.7]

LAST_EXEC_NS = {"d1": None, "d2": None}


def _mk_nc():
    import concourse.bacc as bacc

    return bacc.Bacc(
        "TRN2",
        target_bir_lowering=False,
        debug=False,
        enable_asserts=False,
        num_devices=NCORES,
    )


@functools.cache
def _build_d1():
    """Dispatch 1: projections + sinkhorn attention for one head (= one core).

    out[b] = (q + attn)^T as [DH, N] f32 per batch."""
    import concourse.bass as bass  # noqa: F401
    import concourse.mybir as mybir
    import concourse.tile as tile

    f32 = mybir.dt.float32
    bf16 = mybir.dt.bfloat16
    f8 = mybir.dt.float8e4
    AF = mybir.ActivationFunctionType
    ALU = mybir.AluOpType
    DR = mybir.MatmulPerfMode.DoubleRow

    nc = _mk_nc()
    QT = nc.dram_tensor("QT", [B, D, N], bf16, kind="ExternalInput").ap()
    KT8 = nc.dram_tensor("KT8", [B, 8, 128, N], f8, kind="ExternalInput").ap()
    WQ = nc.dram_tensor("WQ", [128, 8, 128], bf16, kind="ExternalInput").ap()
    WK = nc.dram_tensor("WK", [128, 8, 128], f8, kind="ExternalInput").ap()
    WV = nc.dram_tensor("WV", [128, 8, 128], f8, kind="ExternalInput").ap()
    BQ = nc.dram_tensor("BQ", [128, 1], f32, kind="ExternalInput").ap()
    BK = nc.dram_tensor("BK", [128, 1], f32, kind="ExternalInput").ap()
    BV = nc.dram_tensor("BV", [128, 1], f32, kind="ExternalInput").ap()
    ONES8 = nc.dram_tensor("ONES8", [128, 2, 128], f8, kind="ExternalInput").ap()
    OT = nc.dram_tensor("OT", [B, DH, N], f32, kind="ExternalOutput").ap()

    with tile.TileContext(nc) as tc:
        with (
            tc.tile_pool(name="const", bufs=1) as constp,
            tc.tile_pool(name="io", bufs=3) as iop,
            tc.tile_pool(name="kt", bufs=2) as ktp,
            tc.tile_pool(name="mid", bufs=2) as midp,
            tc.tile_pool(name="et", bufs=2) as etp,
            tc.tile_pool(name="ps_proj", bufs=2, space="PSUM") as ps_proj,
            tc.tile_pool(name="ps_st", bufs=2, space="PSUM") as ps_st,
            tc.tile_pool(name="ps_r", bufs=2, space="PSUM") as ps_r,
            tc.tile_pool(name="ps_ot", bufs=2, space="PSUM") as ps_ot,
        ):
            wq = constp.tile([128, 8, 128], bf16)
            wk = constp.tile([128, 8, 128], f8)
            wv = constp.tile([128, 8, 128], f8)
            bq = constp.tile([128, 1], f32)
            bk = constp.tile([128, 1], f32)
            bv = constp.tile([128, 1], f32)
            ones8 = constp.tile([128, 2, 128], f8)
            nc.sync.dma_start(wq, WQ)
            nc.sync.dma_start(wk, WK)
            nc.sync.dma_start(wv, WV)
            nc.sync.dma_start(bq, BQ)
            nc.sync.dma_start(bk, BK)
            nc.sync.dma_start(bv, BV)
            nc.sync.dma_start(ones8, ONES8)

            def phase_a(b):
                """PE-heavy: projections, S+exp, omega*R + reciprocal."""
                ktc = ktp.tile([128, 8, N], f8, tag="ktc", name=f"ktc{b}")
                nc.sync.dma_start(ktc, KT8[b].rearrange("kc p n -> p kc n"))

                qps = [
                    ps_proj.tile([128, 512], f32, tag="proj", name=f"qps{b}_{i}")
                    for i in range(2)
                ]
                for kc in range(8):
                    qtc = iop.tile([128, N], bf16, tag="qtc", name=f"qtc{b}_{kc}")
                    nc.sync.dma_start(qtc, QT[b, kc * 128 : (kc + 1) * 128, :])
                    for hf in range(2):
                        nc.tensor.matmul(
                            qps[hf],
                            wq[:, kc, :],
                            qtc[:, hf * 512 : (hf + 1) * 512],
                            start=(kc == 0),
                            stop=(kc == 7),
                        )
                # double evacuation: f32 copy for the residual, bf16 for S
                qTf = midp.tile([128, N], f32, tag="qTf", name=f"qTf{b}")
                qT = midp.tile([128, N], bf16, tag="qT", name=f"qT{b}")
                for hf in range(2):
                    hs = slice(hf * 512, (hf + 1) * 512)
                    nc.scalar.activation(qTf[:, hs], qps[hf], AF.Identity, bias=bq)
                    nc.scalar.activation(qT[:, hs], qps[hf], AF.Identity, bias=bq)

                kps = [
                    ps_proj.tile([128, 512], f32, tag="proj", name=f"kps{b}_{i}")
                    for i in range(2)
                ]
                for t in range(4):
                    for hf in range(2):
                        nc.tensor.matmul(
                            kps[hf],
                            wk[:, 2 * t : 2 * t + 2, :],
                            ktc[:, 2 * t : 2 * t + 2, hf * 512 : (hf + 1) * 512],
                            start=(t == 0),
                            stop=(t == 3),
                            perf_mode=DR,
                        )
                kT = midp.tile([128, N], bf16, tag="kT", name=f"kT{b}")
                for hf in range(2):
                    nc.scalar.activation(
                        kT[:, hf * 512 : (hf + 1) * 512],
                        kps[hf],
                        AF.Identity,
                        bias=bk,
                        scale=1.0 / 1024.0,
                    )

                vps = [
                    ps_proj.tile([128, 512], f32, tag="proj", name=f"vps{b}_{i}")
                    for i in range(2)
                ]
                for t in range(4):
                    for hf in range(2):
                        nc.tensor.matmul(
                            vps[hf],
                            wv[:, 2 * t : 2 * t + 2, :],
                            ktc[:, 2 * t : 2 * t + 2, hf * 512 : (hf + 1) * 512],
                            start=(t == 0),
                            stop=(t == 3),
                            perf_mode=DR,
                        )
                vT = midp.tile([128, N], bf16, tag="vT", name=f"vT{b}")
                for hf in range(2):
                    nc.scalar.activation(
                        vT[:, hf * 512 : (hf + 1) * 512],
                        vps[hf],
                        AF.Identity,
                        bias=bv,
                        scale=1.0 / 32.0,
                    )
                # v = vT^T via DMA xbar transpose, issued from the scalar queue
                # so it does not contend with sync-queue input loads
                v = midp.tile([128, 8, 128], bf16, tag="v", name=f"v{b}")
                for mt in range(8):
                    nc.scalar.dma_start_transpose(
                        v[:, mt, :], vT[:, mt * 128 : (mt + 1) * 128]
                    )

                ET = etp.tile([128, 8, N], f8, tag="ET", name=f"ET{b}")
                for mt in range(8):
                    for hf in range(2):
                        sps = ps_st.tile(
                            [128, 512], f32, tag="st", name=f"sps{b}_{mt}_{hf}"
                        )
                        nc.tensor.matmul(
                            sps,
                            kT[:, mt * 128 : (mt + 1) * 128],
                            qT[:, hf * 512 : (hf + 1) * 512],
                            start=True,
                            stop=True,
                        )
                        nc.scalar.activation(
                            ET[:, mt, hf * 512 : (hf + 1) * 512], sps, AF.Exp
                        )

                # omega*R broadcast to every PSUM partition (all-ones DR mm)
                abc = midp.tile([128, N], f32, tag="abc", name=f"abc{b}")
                for hf in range(2):
                    rps = ps_r.tile([128, 512], f32, tag="r", name=f"rps{b}_{hf}")
                    for t in range(4):
                        nc.tensor.matmul(
                            rps,
                            ones8,
                            ET[:, 2 * t : 2 * t + 2, hf * 512 : (hf + 1) * 512],
                            start=(t == 0),
                            stop=(t == 3),
                            perf_mode=DR,
                        )
                    nc.vector.reciprocal_approx_fast(
                        abc[:, hf * 512 : (hf + 1) * 512], rps
                    )
                return dict(qTf=qTf, qT=qT, abc=abc, ET=ET, v=v)

            def phase_b(s, b):
                """DVE chain + attention matmuls + residual + output DMA."""
                ET, abc, v, qTf = s["ET"], s["abc"], s["v"], s["qTf"]
                cw = midp.tile([128, 8], f32, tag="cw", name=f"cw{b}")
                for mt in range(8):
                    nc.vector.scalar_tensor_tensor(
                        out=ET[:, mt, :],
                        in0=ET[:, mt, :],
                        scalar=1.0,
                        in1=abc,
                        op0=ALU.mult,
                        op1=ALU.mult,
                        accum_out=cw[:, mt : mt + 1],
                    )
                wrec = midp.tile([128, 8], f32, tag="wrec", name=f"wrec{b}")
                nc.vector.reciprocal_approx_fast(wrec, cw)

                vv = midp.tile([128, 8, 128], f8, tag="vv", name=f"vv{b}")
                for mt in range(8):
                    nc.vector.tensor_scalar(
                        out=vv[:, mt, :],
                        in0=v[:, mt, :],
                        scalar1=wrec[:, mt : mt + 1],
                        scalar2=1.0 / OMEGA,
                        op0=ALU.mult,
                        op1=ALU.mult,
                    )

                ops_ = [
                    ps_ot.tile([128, 512], f32, tag="ot", name=f"ot{b}_{i}")
                    for i in range(2)
                ]
                for t in range(4):
                    for hf in range(2):
                        nc.tensor.matmul(
                            ops_[hf],
                            vv[:, 2 * t : 2 * t + 2, :],
                            ET[:, 2 * t : 2 * t + 2, hf * 512 : (hf + 1) * 512],
                            start=(t == 0),
                            stop=(t == 3),
                            perf_mode=DR,
                        )
                ofin = midp.tile([128, N], f32, tag="ofin", name=f"ofin{b}")
                for hf in range(2):
                    hs = slice(hf * 512, (hf + 1) * 512)
                    nc.vector.scalar_tensor_tensor(
                        out=ofin[:, hs],
                        in0=ops_[hf],
                        scalar=float(MU) * OMEGA,
                        in1=qTf[:, hs],
                        op0=ALU.mult,
                        op1=ALU.add,
                    )
                nc.sync.dma_start(OT[b], ofin)

            # software pipeline: A(0) A(1) B(0) A(2) B(1) A(3) B(2) B(3)
            state = []
            for b in range(B):
                state.append(phase_a(b))
                if b >= 1:
                    phase_b(state[b - 1], b - 1)
            phase_b(state[B - 1], B - 1)

    nc.compile()
    return nc


@functools.cache
def _build_d2():
    """Dispatch 2 (transposed): LN0 -> fc_o(+relu, residual) -> LN1 on a
    [1024 d, 512 n] column slab per core. LN stats via broadcast all-ones
    matmuls; no on-chip transposes."""
    import concourse.mybir as mybir
    import concourse.tile as tile

    f32 = mybir.dt.float32
    bf16 = mybir.dt.bfloat16
    AF = mybir.ActivationFunctionType
    ALU = mybir.AluOpType

    CS = 4
    NC_ = 512 // CS  # column tile width

    nc = _mk_nc()
    XIN = nc.dram_tensor("XIN", [8, 128, 512], bf16, kind="ExternalInput").ap()
    WOT = nc.dram_tensor("WOT", [128, 8, D], bf16, kind="ExternalInput").ap()
    ONESB = nc.dram_tensor("ONESB", [128, 128], bf16, kind="ExternalInput").ap()
    BO = nc.dram_tensor("BO", [128, 8], f32, kind="ExternalInput").ap()
    G0 = nc.dram_tensor("G0", [128, 8], f32, kind="ExternalInput").ap()
    BE0 = nc.dram_tensor("BE0", [128, 8], f32, kind="ExternalInput").ap()
    G1 = nc.dram_tensor("G1", [128, 8], f32, kind="ExternalInput").ap()
    BE1 = nc.dram_tensor("BE1", [128, 8], f32, kind="ExternalInput").ap()
    OUT2 = nc.dram_tensor("OUT2", [8, 128, 512], f32, kind="ExternalOutput").ap()

    with tile.TileContext(nc) as tc:
        with (
            tc.tile_pool(name="const", bufs=1) as constp,
            tc.tile_pool(name="work", bufs=2) as wp,
            tc.tile_pool(name="small", bufs=2) as sp,
            tc.tile_pool(name="ps_mm", bufs=3, space="PSUM") as ps_mm,
            tc.tile_pool(name="ps_s", bufs=4, space="PSUM") as ps_s,
        ):
            wot = constp.tile([128, 8, D], bf16)
            onesb = constp.tile([128, 128], bf16)
            boc = constp.tile([128, 8], f32)
            g0c = constp.tile([128, 8], f32)
            be0c = constp.tile([128, 8], f32)
            g1c = constp.tile([128, 8], f32)
            be1c = constp.tile([128, 8], f32)
            nc.sync.dma_start(wot, WOT)
            nc.sync.dma_start(onesb, ONESB)
            nc.sync.dma_start(boc, BO)
            nc.sync.dma_start(g0c, G0)
            nc.sync.dma_start(be0c, BE0)
            nc.sync.dma_start(g1c, G1)
            nc.sync.dma_start(be1c, BE1)

            def ln_tr(x_in, gcol, becol, out_dtype, tag, uid):
                """x_in [128, 8, NC_]: LN over d = (partition, chunk).
                Sums via broadcast all-ones matmuls (every PSUM partition gets
                the full column sum)."""
                ps1 = ps_s.tile([128, NC_], f32, tag="s", name=f"ps1_{uid}")
                for dc in range(8):
                    nc.tensor.matmul(
                        ps1,
                        onesb,
                        x_in[:, dc, :],
                        start=(dc == 0),
                        stop=(dc == 7),
                    )
                xsq = sp.tile([128, 8, NC_], bf16, tag="xsq", name=f"xsq_{uid}")
                nc.vector.tensor_tensor(xsq, x_in, x_in, ALU.mult)
                ps2 = ps_s.tile([128, NC_], f32, tag="s", name=f"ps2_{uid}")
                for dc in range(8):
                    nc.tensor.matmul(
                        ps2,
                        onesb,
                        xsq[:, dc, :],
                        start=(dc == 0),
                        stop=(dc == 7),
                    )
                mbc = sp.tile([128, NC_], f32, tag="mbc", name=f"mbc_{uid}")
                nc.vector.tensor_scalar(
                    out=mbc, in0=ps1, scalar1=1.0 / D, scalar2=None, op0=ALU.mult
                )
                m2 = sp.tile([128, NC_], f32, tag="m2", name=f"m2_{uid}")
                nc.vector.tensor_tensor(m2, mbc, mbc, ALU.mult)
                v1 = sp.tile([128, NC_], f32, tag="v1", name=f"v1_{uid}")
                nc.vector.tensor_scalar(
                    out=v1,
                    in0=ps2,
                    scalar1=1.0 / D,
                    scalar2=LN_EPS,
                    op0=ALU.mult,
                    op1=ALU.add,
                )
                var = sp.tile([128, NC_], f32, tag="var", name=f"var_{uid}")
                nc.vector.tensor_tensor(var, v1, m2, ALU.subtract)
                sq = sp.tile([128, NC_], f32, tag="sq", name=f"sq_{uid}")
                nc.scalar.activation(sq, var, AF.Sqrt)
                rstd = sp.tile([128, NC_], f32, tag="rstd", name=f"rstd_{uid}")
                nc.vector.reciprocal_approx_fast(rstd, sq)
                mb16 = sp.tile([128, NC_], bf16, tag="mb16", name=f"mb16_{uid}")
                nc.scalar.activation(mb16, mbc, AF.Copy)
                rb16 = sp.tile([128, NC_], bf16, tag="rb16", name=f"rb16_{uid}")
                nc.scalar.activation(rb16, rstd, AF.Copy)

                out = wp.tile([128, 8, NC_], out_dtype, tag=tag, name=f"{tag}_{uid}")
                for dc in range(8):
                    xc = sp.tile([128, NC_], bf16, tag="xc", name=f"xc_{uid}_{dc}")
                    nc.vector.tensor_tensor(xc, x_in[:, dc, :], mb16, ALU.subtract)
                    xr = sp.tile([128, NC_], bf16, tag="xr", name=f"xr_{uid}_{dc}")
                    nc.vector.tensor_tensor(xr, xc, rb16, ALU.mult)
                    nc.scalar.activation(
                        out[:, dc, :],
                        xr,
                        AF.Identity,
                        scale=gcol[:, dc : dc + 1],
                        bias=becol[:, dc : dc + 1],
                    )
                return out

            def phase_a(i):
                ns = slice(i * NC_, (i + 1) * NC_)
                x = wp.tile([128, 8, NC_], bf16, tag="x", name=f"x{i}")
                nc.sync.dma_start(x, XIN[:, :, ns].rearrange("dc p n -> p dc n"))
                oln = ln_tr(x, g0c, be0c, bf16, "oln", f"a{i}")
                return oln

            def phase_bm(oln, i):
                pss = []
                for do in range(8):
                    ps = ps_mm.tile([128, NC_], f32, tag="mm", name=f"mm{i}_{do}")
                    for di in range(8):
                        nc.tensor.matmul(
                            ps,
                            wot[:, di, do * 128 : (do + 1) * 128],
                            oln[:, di, :],
                            start=(di == 0),
                            stop=(di == 7),
                        )
                    pss.append(ps)
                return pss

            def phase_c(oln, pss, i):
                ns = slice(i * NC_, (i + 1) * NC_)
                t1 = wp.tile([128, 8, NC_], bf16, tag="t1", name=f"t1_{i}")
                for do in range(8):
                    nc.scalar.activation(
                        t1[:, do, :], pss[do], AF.Relu, bias=boc[:, do : do + 1]
                    )
                o2 = wp.tile([128, 8, NC_], bf16, tag="o2", name=f"o2_{i}")
                nc.vector.tensor_tensor(o2, t1, oln, ALU.add)
                o3 = ln_tr(o2, g1c, be1c, f32, "o3", f"c{i}")
                nc.sync.dma_start(OUT2[:, :, ns].rearrange("dc p n -> p dc n"), o3)

            # pipeline: A0 A1 B0 A2 C0 B1 A3 C1 B2 C2 B3 C3
            olns = [None] * CS
            pss = [None] * CS
            olns[0] = phase_a(0)
            olns[1] = phase_a(1)
            pss[0] = phase_bm(olns[0], 0)
            for i in range(2, CS):
                olns[i] = phase_a(i)
                phase_c(olns[i - 2], pss[i - 2], i - 2)
                pss[i - 1] = phase_bm(olns[i - 1], i - 1)
            phase_c(olns[CS - 2], pss[CS - 2], CS - 2)
            pss[CS - 1] = phase_bm(olns[CS - 1], CS - 1)
            phase_c(olns[CS - 1], pss[CS - 1], CS - 1)

    nc.compile()
    return nc


def _run(nc, in_maps, trace=False):
    from concourse.bass_utils import run_bass_kernel_spmd

    return run_bass_kernel_spmd(nc, in_maps, list(range(NCORES)), trace=trace)


def kernel(**inputs):
    trace = bool(int(__import__("os").environ.get("KERNEL_TRACE", "0")))
    f32 = np.float32
    bf16 = ml_dtypes.bfloat16
    f8 = ml_dtypes.float8_e4m3fn
    Q = np.ascontiguousarray(inputs["Q"], dtype=f32)
    K = np.ascontiguousarray(inputs["K"], dtype=f32)
    Wq, Wk, Wv, Wo = (np.asarray(inputs[k], f32) for k in ("Wq", "Wk", "Wv", "Wo"))
    bq, bk, bv, bo = (np.asarray(inputs[k], f32) for k in ("bq", "bk", "bv", "bo"))
    g0, be0, g1, be1 = (np.asarray(inputs[k], f32) for k in ("g0", "be0", "g1", "be1"))

    QT = np.ascontiguousarray(Q.transpose(0, 2, 1)).astype(bf16)
    KT8 = (
        np.ascontiguousarray(K.transpose(0, 2, 1)).reshape(B, 8, 128, N).astype(f8)
    )
    ones8 = np.full((128, 2, 128), OMEGA, dtype=f8)

    in_maps = []
    for h in range(H):
        hs = slice(h * DH, (h + 1) * DH)
        wqh = np.ascontiguousarray(
            Wq[:, hs].reshape(8, 128, 128).transpose(1, 0, 2)
        ).astype(bf16)
        wkh = np.ascontiguousarray(
            (Wk[:, hs] * 32.0).reshape(8, 128, 128).transpose(1, 0, 2)
        ).astype(f8)
        wvh = np.ascontiguousarray(
            (Wv[:, hs] * 32.0).reshape(8, 128, 128).transpose(1, 0, 2)
        ).astype(f8)
        in_maps.append(
            {
                "QT": QT,
                "KT8": KT8,
                "WQ": wqh,
                "WK": wkh,
                "WV": wvh,
                "BQ": bq[hs].reshape(128, 1).astype(f32),
                "BK": (bk[hs] * SCALE).reshape(128, 1).astype(f32),
                "BV": bv[hs].reshape(128, 1).astype(f32),
                "ONES8": ones8,
            }
        )

    r1 = _run(_build_d1(), in_maps, trace=trace)
    LAST_EXEC_NS["d1"] = r1.exec_time_ns

    # ---- host reshard: per-head [B, DH, N] f32 -> per-core [8, 128, 512] ----
    # core c <-> (batch c//2, n-half c%2); chunk dim = head index
    OTall = np.stack([r1.results[h]["OT"] for h in range(H)])  # [8, B, 128, N]
    wot_in = np.ascontiguousarray(
        Wo.reshape(8, 128, D).transpose(1, 0, 2)
    ).astype(bf16)
    onesb = np.ones((128, 128), dtype=bf16)
    col = lambda z: np.ascontiguousarray(z.reshape(8, 128).T, dtype=f32)
    boc, g0c, be0c, g1c, be1c = col(bo), col(g0), col(be0), col(g1), col(be1)
    in_maps2 = []
    for c in range(NCORES):
        b, nh = c // 2, c % 2
        xin = np.ascontiguousarray(
            OTall[:, b, :, nh * 512 : (nh + 1) * 512]
        ).astype(bf16)
        in_maps2.append(
            {
                "XIN": xin,
                "WOT": wot_in,
                "ONESB": onesb,
                "BO": boc,
                "G0": g0c,
                "BE0": be0c,
                "G1": g1c,
                "BE1": be1c,
            }
        )
    r2 = _run(_build_d2(), in_maps2, trace=trace)
    LAST_EXEC_NS["d2"] = r2.exec_time_ns

    # ---- host unshard: [8 dc, 128 p, 512 n] -> [n, d] rows of O ----
    out = np.empty((B, N, D), dtype=f32)
    for c in range(NCORES):
        b, nh = c // 2, c % 2
        slab = r2.results[c]["OUT2"]  # [8, 128, 512] f32
        out[b, nh * 512 : (nh + 1) * 512, :] = (
            slab.transpose(2, 0, 1).reshape(512, D)
        )
    return out
